# revision 1
# baseline (speedup 1.0000x reference)
"""Trainium2 kernel for retrieval_knn (CML): batched embedding-distance computation.

Approach (per core, data-parallel over batch):
  dist(b,k) = ||u_b - (Wi[i]+Wart[a]+Walb[l])/3||^2  for 64 negatives + 1 positive.

The dominant cost is ~50MB/core of random 512B row gathers. dma_gather (bulk
SWDGE gather) only takes int16 indices (32768-row windows), so we run two
stages through a DRAM staging buffer:
  stage 1: sort each table's 32768 row-indices; gather window-bucketed
           chunks with dma_gather (exact counts, 128-multiples) plus the
           per-window remainders with indirect_dma_start (int32, no window
           limit); write everything into staging[128, 256, 128] in
           per-partition-contiguous layout (cheap large-descriptor writes).
  stage 2: one dma_gather per (b-chunk, k-half, table) from the 32768-row
           staging view, with indices in (k,b) order so the output tile has
           partition == batch row; then DVE/ACT compute:
           S = Xi+Xa+Xl; S -= 3u (broadcast); S ^= 2; reduce over D; /9;
           min over k.
Uniform SPMD program: per-window main capacities use the min count across the
8 cores, so leftover totals are core-invariant (no dedup => per-core streams
are exactly 32768 rows).
"""
import sys

sys.path.insert(0, "/opt/trn_rl_repo")

import numpy as np

N_USERS, N_ITEMS, N_ARTISTS, N_ALBUMS = 100000, 500000, 100000, 200000
D = 128
B, K = 4096, 64
NCORES = 8
BL = B // NCORES          # 512 batch rows per core
S = BL * K                # 32768 negative slots per core
W = 32768                 # dma_gather index window (rows)
COLS = 256                # staging columns (128*256 = 32768 rows)
SUBCOLS = 16              # max columns (x128 rows) per stage-1 dma_gather call
NQ = 4                    # SWDGE queues

_compiled = {}


def _make_idx_sb(idx_flat, dtype):
    """Wrap a flat index list into the [128, n/16] SWDGE layout (16-partition
    wrap, replicated for the 8 Q7 cores)."""
    n = idx_flat.shape[0]
    blk = np.ascontiguousarray(idx_flat.reshape(n // 16, 16).T).astype(dtype)
    return np.tile(blk, (8, 1))


def _plan_table(vals_all, n_rows):
    """Plan stage-1 for one table across all cores.

    vals_all: [NCORES, S] int64 row indices.
    Returns (calls, Lg, per_core) where
      calls: list of (window, ncols, col_off, idx_off) stage-1 dma_gather calls
      Lg: number of 128-row leftover (indirect) groups
      per_core: list of dicts with main16 [128, main_cols*8] int16,
                left32 [128, Lg] int32, row_id [S] int32 (slot -> staging row)
    """
    n_win = (n_rows + W - 1) // W
    orders = [np.argsort(v, kind="stable") for v in vals_all]
    svs = [v[o] for v, o in zip(vals_all, orders)]
    bounds = [np.searchsorted(sv, np.arange(n_win + 1) * W) for sv in svs]
    nw = np.stack([b[1:] - b[:-1] for b in bounds])          # [NCORES, n_win]
    Cw = (nw.min(axis=0) // 128).astype(np.int64)
    Cw[Cw < 2] = 0                                           # avoid 128-row calls

    calls = []
    col = 0
    idx_off = 0
    for w in range(n_win):
        c_rem = int(Cw[w])
        while c_rem > 0:
            take = c_rem if c_rem <= SUBCOLS + 1 else SUBCOLS
            calls.append((w, take, col, idx_off))
            col += take
            idx_off += take * 8
            c_rem -= take
    main_cols = col
    Lg = COLS - main_cols
    assert Lg >= 0

    per_core = []
    for c in range(NCORES):
        sv, order, bnd = svs[c], orders[c], bounds[c]
        main16 = np.zeros((128, main_cols * 8), np.int16)
        row_id_sorted = np.empty(S, np.int32)
        left_parts = []
        left_pos_parts = []
        for w in range(n_win):
            lo, hi = int(bnd[w]), int(bnd[w + 1])
            nmain = int(Cw[w]) * 128
            seg = sv[lo:lo + nmain]
            if nmain:
                i = np.arange(nmain)
                # row-id for sorted positions of the main part: the col is
                # call-local (each sub-call re-starts its interleave)
                done = 0
                for (w2, take, coff, ioff) in calls:
                    if w2 != w:
                        continue
                    n_call = take * 128
                    sub = seg[done:done + n_call]
                    il = np.arange(n_call)
                    row_id_sorted[lo + done + il] = (il % 128) * COLS + coff + il // 128
                    main16[:, ioff:ioff + take * 8] = _make_idx_sb(
                        (sub - w * W).astype(np.int64), np.int16)
                    done += n_call
            tail = sv[lo + nmain:hi]
            left_parts.append(tail)
            left_pos_parts.append(np.arange(lo + nmain, hi))
        left = np.concatenate(left_parts) if left_parts else np.empty(0, np.int64)
        left_pos = np.concatenate(left_pos_parts) if left_pos_parts else np.empty(0, np.int64)
        assert left.shape[0] == Lg * 128, (left.shape, Lg)
        j = np.arange(Lg * 128)
        row_id_sorted[left_pos] = (j % 128) * COLS + main_cols + j // 128
        left32 = np.zeros((128, max(Lg, 1)), np.int32)
        if Lg:
            left32[:, :] = left.reshape(Lg, 128).T
        row_id = np.empty(S, np.int32)
        row_id[order] = row_id_sorted
        per_core.append(dict(main16=main16, left32=left32, row_id=row_id))
    return calls, Lg, per_core


def _build_program(meta):
    """Build + compile the SPMD bass program for the given stage-1 plan shapes.

    meta: tuple of (calls tuple, Lg) per table, hashable.
    """
    import concourse.bass as bass
    import concourse.bacc as bacc
    import concourse.mybir as mybir
    import concourse.tile as tile

    f32 = mybir.dt.float32
    i16 = mybir.dt.int16
    i32 = mybir.dt.int32

    tables = [("i", N_ITEMS), ("a", N_ARTISTS), ("l", N_ALBUMS)]
    plan = {t: meta[ti] for ti, (t, _) in enumerate(tables)}

    nc = bacc.Bacc(None, target_bir_lowering=False, debug=False,
                   num_swdge_queues=NQ)
    qctr = [0]

    def q():
        qctr[0] = (qctr[0] + 1) % NQ
        return qctr[0]

    with tile.TileContext(nc) as tc:
        with tc.tile_pool(name="dram", bufs=1, space="DRAM") as dram:
            Wt = {
                "i": dram.tile([N_ITEMS, D], f32, kind="ExternalInput", name="Wi"),
                "a": dram.tile([N_ARTISTS, D], f32, kind="ExternalInput", name="Wa"),
                "l": dram.tile([N_ALBUMS, D], f32, kind="ExternalInput", name="Wl"),
            }
            Wu = dram.tile([N_USERS, D], f32, kind="ExternalInput", name="Wu")
            g1idx = {}
            lidx = {}
            s2idx = {}
            for t, _n in tables:
                calls, Lg = plan[t]
                mc = sum(take for (_w, take, _c, _i) in calls)
                g1idx[t] = dram.tile([128, max(mc * 8, 1)], i16,
                                     kind="ExternalInput", name=f"g1idx_{t}")
                lidx[t] = dram.tile([128, max(Lg, 1)], i32,
                                    kind="ExternalInput", name=f"lidx_{t}")
                s2idx[t] = dram.tile([8, 128, 256], i16,
                                     kind="ExternalInput", name=f"s2idx_{t}")
            uidx = dram.tile([BL, 1], i32, kind="ExternalInput", name="uidx")
            pidx = {t: dram.tile([BL, 1], i32, kind="ExternalInput",
                                 name=f"pidx_{t}") for t, _n in tables}
            stag = {t: dram.tile([128, COLS, D], f32, kind="Internal",
                                 name=f"stag_{t}") for t, _n in tables}
            pos_d = dram.tile([BL, 1], f32, kind="ExternalOutput", name="pos_d")
            dneg = dram.tile([BL, K], f32, kind="ExternalOutput", name="dneg")
            clo = dram.tile([BL, 1], f32, kind="ExternalOutput", name="clo")

            with (
                tc.tile_pool(name="g1", bufs=6) as g1p,
                tc.tile_pool(name="idxp", bufs=4) as idxp,
                tc.tile_pool(name="x2", bufs=2) as x2p,
                tc.tile_pool(name="small", bufs=4) as smallp,
            ):
                # ---------------- stage 1 ----------------
                for t, n_rows in tables:
                    calls, Lg = plan[t]
                    for (w, take, coff, ioff) in calls:
                        n_call = take * 128
                        it = idxp.tile([128, take * 8], i16, tag="i1")
                        nc.sync.dma_start(out=it[:], in_=g1idx[t][:, ioff:ioff + take * 8])
                        X = g1p.tile([128, take * D], f32, tag="g1")
                        wlen = min(W, n_rows - w * W)
                        nc.gpsimd.dma_gather(
                            out_ap=X[:].rearrange("p (c d) -> p c d", d=D),
                            in_ap=Wt[t][w * W:w * W + wlen, :],
                            idxs_ap=it[:],
                            num_idxs=n_call, num_idxs_reg=n_call, elem_size=D,
                            single_packet=False, queue_num=q(),
                        )
                        nc.sync.dma_start(
                            out=stag[t][:, coff:coff + take, :],
                            in_=X[:].rearrange("p (c d) -> p c d", d=D),
                        )
                    if Lg:
                        lt = idxp.tile([128, Lg], i32, tag="l1")
                        nc.sync.dma_start(out=lt[:], in_=lidx[t][:, :Lg])
                        mc = sum(take for (_w, take, _c, _i) in calls)
                        for g in range(Lg):
                            Xl_ = g1p.tile([128, D], f32, tag="lg")
                            nc.gpsimd.indirect_dma_start(
                                out=Xl_[:], out_offset=None, in_=Wt[t][:],
                                in_offset=bass.IndirectOffsetOnAxis(
                                    ap=lt[:, g:g + 1], axis=0),
                            )
                            nc.sync.dma_start(
                                out=stag[t][:, mc + g:mc + g + 1, :],
                                in_=Xl_[:, None, :],
                            )

                # ---------------- stage 2 ----------------
                stag_flat = {t: stag[t][:].rearrange("p c d -> (p c) d")
                             for t, _n in tables}
                inv9 = 1.0 / 9.0
                for cb in range(4):
                    r0 = cb * 128
                    # user rows for this b-chunk (partition == b)
                    ut = smallp.tile([128, 1], i32, tag="uti")
                    nc.sync.dma_start(out=ut[:], in_=uidx[r0:r0 + 128, :])
                    U = smallp.tile([128, D], f32, tag="U")
                    nc.gpsimd.indirect_dma_start(
                        out=U[:], out_offset=None, in_=Wu[:],
                        in_offset=bass.IndirectOffsetOnAxis(ap=ut[:, :1], axis=0),
                    )
                    U3 = smallp.tile([128, D], f32, tag="U3")
                    nc.scalar.mul(U3[:], U[:], 3.0)

                    # ---- positives for this b-chunk ----
                    Ps = []
                    for t, _n in tables:
                        pt = smallp.tile([128, 1], i32, tag=f"pti{t}")
                        nc.sync.dma_start(out=pt[:], in_=pidx[t][r0:r0 + 128, :])
                        P = smallp.tile([128, D], f32, tag=f"P{t}")
                        nc.gpsimd.indirect_dma_start(
                            out=P[:], out_offset=None, in_=Wt[t][:],
                            in_offset=bass.IndirectOffsetOnAxis(ap=pt[:, :1], axis=0),
                        )
                        Ps.append(P)
                    nc.vector.tensor_add(out=Ps[0][:], in0=Ps[0][:], in1=Ps[1][:])
                    nc.vector.tensor_add(out=Ps[0][:], in0=Ps[0][:], in1=Ps[2][:])
                    nc.vector.tensor_tensor(out=Ps[0][:], in0=Ps[0][:], in1=U3[:],
                                            op=mybir.AluOpType.subtract)
                    nc.scalar.square(Ps[0][:], Ps[0][:])
                    pd = smallp.tile([128, 1], f32, tag="pd")
                    nc.vector.tensor_reduce(out=pd[:], in_=Ps[0][:],
                                            axis=mybir.AxisListType.X,
                                            op=mybir.AluOpType.add)
                    nc.scalar.mul(pd[:], pd[:], inv9)
                    nc.sync.dma_start(out=pos_d[r0:r0 + 128, :], in_=pd[:])

                    # ---- negatives: two k-halves ----
                    km = []
                    for kh in range(2):
                        Xs = []
                        for ti, (t, _n) in enumerate(tables):
                            st = idxp.tile([128, 256], i16, tag=f"s2{t}")
                            nc.sync.dma_start(out=st[:], in_=s2idx[t][cb * 2 + kh])
                            Xt = x2p.tile([128, 32 * D], f32, tag=f"X{t}")
                            nc.gpsimd.dma_gather(
                                out_ap=Xt[:].rearrange("p (c d) -> p c d", d=D),
                                in_ap=stag_flat[t],
                                idxs_ap=st[:],
                                num_idxs=4096, num_idxs_reg=4096, elem_size=D,
                                single_packet=False, queue_num=q(),
                            )
                            Xs.append(Xt)
                        Xi = Xs[0]
                        nc.vector.tensor_add(out=Xi[:], in0=Xi[:], in1=Xs[1][:])
                        nc.vector.tensor_add(out=Xi[:], in0=Xi[:], in1=Xs[2][:])
                        Xi3 = Xi[:].rearrange("p (c d) -> p c d", d=D)
                        U3b = U3[:, None, :].to_broadcast([128, 32, D])
                        nc.vector.tensor_tensor(out=Xi3, in0=Xi3, in1=U3b,
                                                op=mybir.AluOpType.subtract)
                        nc.scalar.square(Xi[:], Xi[:])
                        dist = smallp.tile([128, 32], f32, tag="dist")
                        nc.vector.tensor_reduce(out=dist[:], in_=Xi3,
                                                axis=mybir.AxisListType.X,
                                                op=mybir.AluOpType.add)
                        nc.scalar.mul(dist[:], dist[:], inv9)
                        nc.sync.dma_start(
                            out=dneg[r0:r0 + 128, kh * 32:(kh + 1) * 32],
                            in_=dist[:])
                        kmh = smallp.tile([128, 1], f32, tag=f"km{kh}")
                        nc.vector.tensor_reduce(out=kmh[:], in_=dist[:],
                                                axis=mybir.AxisListType.X,
                                                op=mybir.AluOpType.min)
                        km.append(kmh)
                    nc.vector.tensor_tensor(out=km[0][:], in0=km[0][:],
                                            in1=km[1][:], op=mybir.AluOpType.min)
                    nc.sync.dma_start(out=clo[r0:r0 + 128, :], in_=km[0][:])

    nc.compile()
    names = dict(
        Wt={t: Wt[t].name for t, _ in tables}, Wu=Wu.name,
        g1idx={t: g1idx[t].name for t, _ in tables},
        lidx={t: lidx[t].name for t, _ in tables},
        s2idx={t: s2idx[t].name for t, _ in tables},
        uidx=uidx.name, pidx={t: pidx[t].name for t, _ in tables},
        pos_d=pos_d.name, dneg=dneg.name, clo=clo.name,
    )
    return nc, names


def kernel(user_positive_items_pairs, pos_artists, pos_albums, neg_samples,
           neg_artists, neg_albums, titles, titles_len, Wu, Wi, Wart, Walb):
    from concourse.bass_utils import run_bass_kernel_spmd

    Wu = np.ascontiguousarray(np.asarray(Wu, dtype=np.float32))
    Wi = np.ascontiguousarray(np.asarray(Wi, dtype=np.float32))
    Wart = np.ascontiguousarray(np.asarray(Wart, dtype=np.float32))
    Walb = np.ascontiguousarray(np.asarray(Walb, dtype=np.float32))
    pairs = np.asarray(user_positive_items_pairs, dtype=np.int64)
    pos_artists = np.asarray(pos_artists, dtype=np.int64)
    pos_albums = np.asarray(pos_albums, dtype=np.int64)
    neg_samples = np.asarray(neg_samples, dtype=np.int64)
    neg_artists = np.asarray(neg_artists, dtype=np.int64)
    neg_albums = np.asarray(neg_albums, dtype=np.int64)

    tbl_vals = {
        "i": neg_samples.reshape(NCORES, S),
        "a": neg_artists.reshape(NCORES, S),
        "l": neg_albums.reshape(NCORES, S),
    }
    tbl_rows = {"i": N_ITEMS, "a": N_ARTISTS, "l": N_ALBUMS}
    tbl_arr = {"i": Wi, "a": Wart, "l": Walb}

    plans = {}
    for t in ("i", "a", "l"):
        plans[t] = _plan_table(tbl_vals[t], tbl_rows[t])

    meta = tuple((tuple(plans[t][0]), plans[t][1]) for t in ("i", "a", "l"))
    if meta not in _compiled:
        _compiled[meta] = _build_program(meta)
    nc, names = _compiled[meta]

    # stage-2 index arrays per core: (k,b) order so output partition == b
    in_maps = []
    for c in range(NCORES):
        m = {
            names["Wt"]["i"]: Wi, names["Wt"]["a"]: Wart,
            names["Wt"]["l"]: Walb, names["Wu"]: Wu,
        }
        for t in ("i", "a", "l"):
            calls, Lg, per_core = plans[t]
            pc = per_core[c]
            m[names["g1idx"][t]] = pc["main16"] if pc["main16"].shape[1] else \
                np.zeros((128, 1), np.int16)
            m[names["lidx"][t]] = pc["left32"]
            row_id = pc["row_id"]
            s2 = np.empty((8, 128, 256), np.int16)
            for cb in range(4):
                for kh in range(2):
                    j = np.arange(4096)
                    slots = (cb * 128 + j % 128) * K + kh * 32 + j // 128
                    s2[cb * 2 + kh] = _make_idx_sb(row_id[slots].astype(np.int64),
                                                   np.int16)
            m[names["s2idx"][t]] = s2
        m[names["uidx"]] = pairs[c * BL:(c + 1) * BL, 0].astype(np.int32).reshape(BL, 1)
        m[names["pidx"]["i"]] = pairs[c * BL:(c + 1) * BL, 1].astype(np.int32).reshape(BL, 1)
        m[names["pidx"]["a"]] = pos_artists[c * BL:(c + 1) * BL].astype(np.int32).reshape(BL, 1)
        m[names["pidx"]["l"]] = pos_albums[c * BL:(c + 1) * BL].astype(np.int32).reshape(BL, 1)
        in_maps.append(m)

    res = run_bass_kernel_spmd(nc, in_maps, core_ids=list(range(NCORES)))
    pos_out = np.concatenate([res.results[c][names["pos_d"]].ravel()
                              for c in range(NCORES)])
    dneg_out = np.concatenate([res.results[c][names["dneg"]]
                               for c in range(NCORES)], axis=0)
    clo_out = np.concatenate([res.results[c][names["clo"]].ravel()
                              for c in range(NCORES)])
    return (pos_out.astype(np.float32), dneg_out.astype(np.float32),
            clo_out.astype(np.float32))
